# revision 29
# baseline (speedup 1.0000x reference)
"""Trainium2 Bass kernel for nn_MultiHeadAttention_66202625900642.

Reference semantics (B=2, S=2048, E=1024, H=16 heads, D=64):
    qh = q @ Wq.T + bq   (same k, v)
    head split is a PLAIN RESHAPE (B, S, E) -> (B, H, S, D):
      head h of batch b = rows [128h, 128h+128) of qh[b] reinterpreted
      row-major as a (2048, 64) matrix (scrambled seq index s' = 16r + c).
    causal softmax over s', out @ Wp.T + bp.

Because the head split partitions the *sequence* rows, sharding each batch
into 4 row-blocks of 512 (= 4 heads) is fully local: 8 cores = 2 batches x 4
quarters, zero collectives. Weights are replicated (fp16).

Per-core pipeline (all matmuls fp16, fp32 PSUM accumulation):
  1. projections -> qh/kh/vh fp16 (weights streamed per e-tile)
  2. DRAM round-trip: qh/kh into per-pair [2048, 128] files (2 heads wide),
     vh natural; DMA-transpose pair files back as [128, 2048] = two heads'
     Q_hT/K_hT stacked; vh re-read as [128, 65] V' tiles (ones column -> row
     sums ride along the P^T @ V' matmul).
  3. attention per head pair, both heads' S^T blocks issued to disjoint PE
     row groups (K=64 each -> concurrent on the 128x128 array): one exp per
     psum group on ACT, causal triangles via gpsimd affine_select,
     P^T @ V' accumulates out^T[d, s'] + rowsum into per-chunk PSUM,
     evacuated to SBUF.
  4. normalization: reciprocal of rowsum, PE-broadcast (f32r K=1 matmul),
     fused into the stride-16 rearrange to final-projection layout.
  5. final projection -> y fp32.
"""

import numpy as np

import concourse.bass as bass
import concourse.mybir as mybir
import concourse.tile as tile
from concourse import bacc
from concourse.bass_utils import run_bass_kernel_spmd

F16 = mybir.dt.float16
F32 = mybir.dt.float32
F32R = mybir.dt.float32r
EXP = mybir.ActivationFunctionType.Exp

B, S, E = 2, 2048, 1024
SB = 512                # seq rows per core (= 4 heads)
N_CORES = 8


def build(reps: int = 1, phases: int = 3):
    nc = bacc.Bacc(None, target_bir_lowering=False)

    qT = nc.dram_tensor("qT", [E, SB], F16, kind="ExternalInput")
    kT = nc.dram_tensor("kT", [E, SB], F16, kind="ExternalInput")
    vT = nc.dram_tensor("vT", [E, SB], F16, kind="ExternalInput")
    wqT = nc.dram_tensor("wqT", [E, E], F16, kind="ExternalInput")
    wkT = nc.dram_tensor("wkT", [E, E], F16, kind="ExternalInput")
    wvT = nc.dram_tensor("wvT", [E, E], F16, kind="ExternalInput")
    wpT = nc.dram_tensor("wpT", [E, E], F16, kind="ExternalInput")
    bq = nc.dram_tensor("bq", [1, E], F16, kind="ExternalInput")
    bk = nc.dram_tensor("bk", [1, E], F16, kind="ExternalInput")
    bv = nc.dram_tensor("bv", [1, E], F16, kind="ExternalInput")
    bp = nc.dram_tensor("bp", [1, E], F16, kind="ExternalInput")
    y = nc.dram_tensor("y", [SB, E], F32, kind="ExternalOutput")

    with tile.TileContext(nc) as tc:
        with (
            tc.tile_pool(name="consts", bufs=1) as consts,
            tc.tile_pool(name="wpool", bufs=1) as wpool,
            tc.tile_pool(name="proj", bufs=2) as proj,
            tc.tile_pool(name="attn", bufs=1) as attn,
            tc.tile_pool(name="ptile", bufs=3) as ptile,
            tc.tile_pool(name="ypool", bufs=2) as ypool,
            tc.tile_pool(name="ps", bufs=3, space="PSUM") as ps,
            tc.tile_pool(name="dram", bufs=1, space="DRAM") as dram,
        ):
            # ---- constants -------------------------------------------------
            ones128 = consts.tile([1, 128], F16)
            nc.vector.memset(ones128, 1.0)
            bias_sb = {}
            for nm, t in (("q", bq), ("k", bk), ("v", bv), ("p", bp)):
                b_t = consts.tile([1, E], F16, name=f"bias_{nm}")
                nc.sync.dma_start(out=b_t, in_=t[:, :])
                bias_sb[nm] = b_t

            # ---- weight/activation tiles; q/k loaded now, v/p deferred -----
            w_sb, x_sb, dram_in = {}, {}, {}
            for nm, wt, xt in (("q", wqT, qT), ("k", wkT, kT), ("v", wvT, vT)):
                w_t = wpool.tile([128, 8, E], F16, name=f"w_{nm}")
                x_t = wpool.tile([128, 8, SB], F16, name=f"x_{nm}")
                dram_in[nm] = (wt, xt)
                w_sb[nm], x_sb[nm] = w_t, x_t
            w_p = wpool.tile([128, 8, E], F16, name="w_p")
            w_sb["p"] = w_p
            dram_in["p"] = (wpT, None)

            def load_inputs(nm):
                wt, xt = dram_in[nm]
                wre = wt.ap().rearrange("(t p) f -> p t f", p=128)
                if xt is not None:
                    xre = xt.ap().rearrange("(t p) s -> p t s", p=128)
                    nc.sync.dma_start(out=x_sb[nm], in_=xre)
                for t2 in range(2):
                    nc.sync.dma_start(out=w_sb[nm][:, 4 * t2:4 * t2 + 4],
                                      in_=wre[:, 4 * t2:4 * t2 + 4])

            load_inputs("q")
            load_inputs("k")

            # ---- DRAM scratch ---------------------------------------------
            qkp = [dram.tile([2 * S, 128], F16, name=f"qkp{i}")
                   for i in range(2)]
            vh_d = dram.tile([SB, E], F16)

            for rep in range(reps):
                _body(nc, tc, ps, proj, attn, ptile, ypool,
                      ones128, bias_sb, w_sb, x_sb, qkp, vh_d, y,
                      rep, phases, load_inputs if rep == 0 else None)
    nc.finalize()
    return nc


def _body(nc, tc, ps, proj, attn, ptile, ypool, ones128,
          bias_sb, w_sb, x_sb, qkp, vh_d, y, rep, phases=3,
          load_inputs=None):
    xT2 = attn.tile([128, 8, SB], F16, tag="xT2", name=f"xT2_{rep}")
    if phases < 2:
        nc.vector.memset(xT2[:, 0, 0:1], 0.0)
    _xh_cache = {}

    def project_unit(st, nm, ch):
        # one psum-group of the projection for (seq-tile st, proj nm, chunk ch)
        xh = _xh_cache.get((st, nm))
        if xh is None:
            xh = proj.tile([128, E], F16, tag="xh", name=f"xh_{nm}{st}_{rep}")
            _xh_cache[(st, nm)] = xh
        pp = ps.tile([128, 512], F32, tag="P1", bufs=2, name=f"pp{rep}")
        nc.tensor.matmul(pp, ones128[0:1, :],
                         bias_sb[nm][0:1, bass.ts(ch, 512)],
                         start=True, stop=False)
        for t in range(8):
            nc.tensor.matmul(
                pp,
                x_sb[nm][:, t, bass.ts(st, 128)],
                w_sb[nm][:, t, bass.ts(ch, 512)],
                start=False, stop=(t == 7))
        nc.vector.tensor_copy(xh[:, bass.ts(ch, 512)], pp)
        if ch == 1:
            if nm == "v":
                nc.sync.dma_start(out=vh_d[bass.ts(st, 128), :], in_=xh)
            else:
                tgt = qkp[st // 2]
                base = (0 if nm == "q" else S * 128) + 64 * (st % 2)
                out_ap = bass.AP(
                    tgt.tensor, tgt.offset + base,
                    [[2048, 128], [128, 16], [1, 64]])
                nc.sync.dma_start(
                    out=out_ap, in_=xh.rearrange("r (c d) -> r c d", d=64))

    def project(st):
        for nm in ("q", "k", "v"):
            for ch in range(2):
                project_unit(st, nm, ch)

    def attend_load(pair):
        QKT = ptile.tile([128, 2 * S], F16, tag="QKT", bufs=2,
                         name=f"QKT{pair}_{rep}")
        nc.scalar.dma_start(out=QKT, in_=qkp[pair][:, :], transpose=True)
        return QKT[:, 0:S], QKT[:, S:2 * S]

    def attend(pair, loaded, fillers=(), tail_fill=((), ())):
        QT, KT = loaded
        fillers = list(fillers)
        vps = []
        for half in range(2):
            h = 2 * pair + half
            vp = ptile.tile([128, 16, 65], F16, tag="vp", bufs=4,
                            name=f"vp{h}_{rep}")
            v_src = bass.AP(vh_d.tensor, vh_d.offset + 128 * h * E,
                            [[64, 128], [8192, 16], [1, 64]])
            nc.sync.dma_start(out=vp[:, :, 0:64], in_=v_src)
            nc.vector.memset(vp[:, :, 64:65], 1.0)
            vps.append(vp)

        # per-head SBUF accumulators for out^T (+rowsum row 64)
        osb = [ptile.tile([65, 2048], F32, tag="osb", bufs=3,
                          name=f"osb{2 * pair + half}_{rep}")
               for half in range(2)]

        LAG = 2   # defer V-matmuls 2 groups behind S^T/exp (pt bufs cover it)
        pending = []

        def emit_vmms(ent):
            qc_, js_, pts_, psO_ = ent
            jmax_ = 4 * qc_ + 3
            for half in range(2):
                pt = pts_[half]
                for jj, j in enumerate(js_):
                    o = j - 4 * qc_
                    lo = 0 if o < 0 else 128 * o
                    nc.tensor.matmul(
                        psO_[half][:, lo:],
                        vps[half][:, j, :],
                        pt[:, 512 * jj + lo:512 * jj + 512],
                        start=(j == 0), stop=(j == jmax_))

        for qc in range(4):
            jmax = 4 * qc + 3
            psO = [ps.tile([65, 512], F32, tag="O", bufs=2,
                           name=f"psO{2 * pair + half}_{qc}_{rep}")
                   for half in range(2)]
            for j0 in range(0, jmax + 1, 2):
                js = [j for j in (j0, j0 + 1) if j <= jmax]
                lo0 = max(0, 128 * (js[0] - 4 * qc))
                pts = []
                for half in range(2):
                    psS = ps.tile([128, 1024], F32, tag="S", bufs=2,
                                  name=f"psS{half}_{qc}_{j0}_{rep}")
                    pt = ptile.tile([128, 1024], F16, tag="P", bufs=4,
                                    name=f"pt{half}_{qc}_{j0}_{rep}")
                    r0, r1 = 64 * half, 64 * half + 64
                    for jj, j in enumerate(js):
                        o = j - 4 * qc
                        lo = 0 if o < 0 else 128 * o
                        nc.tensor.matmul(
                            psS[:, 512 * jj + lo:512 * jj + 512],
                            KT[r0:r1, bass.ts(j, 128)],
                            QT[r0:r1, 512 * qc + lo:512 * qc + 512],
                            start=True, stop=True)
                    # one exp per group; stale lead-in cols are never read
                    nc.scalar.activation(pt[:, lo0:], psS[:, lo0:], EXP)
                    pts.append(pt)
                    for jj, j in enumerate(js):
                        o = j - 4 * qc
                        if o >= 0:
                            sl = pts[half][:, 512 * jj + 128 * o:
                                           512 * jj + 128 * o + 128]
                            nc.gpsimd.affine_select(
                                out=sl, in_=sl,
                                pattern=[[1, 128]],
                                compare_op=mybir.AluOpType.is_ge,
                                fill=0.0, base=0, channel_multiplier=-1)
                if fillers:
                    fillers.pop(0)()   # independent PE work while exp runs
                pending.append((qc, js, pts, psO))
                if len(pending) > LAG:
                    emit_vmms(pending.pop(0))
                if fillers:
                    fillers.pop(0)()
            # drain this qc's V-matmuls before evacuating its psO
            while pending:
                emit_vmms(pending.pop(0))
            for half in range(2):
                nc.vector.tensor_copy(osb[half][:, bass.ts(qc, 512)],
                                      psO[half])

            if qc in (1, 3):
                # normalize the finished s' segment [1024*seg, 1024*(seg+1))
                seg = qc // 2
                base = 1024 * seg
                for half in range(2):
                    h = 2 * pair + half
                    recip = ptile.tile([1, 1024], F32, tag="recip", bufs=4,
                                       name=f"recip{h}{seg}_{rep}")
                    nc.vector.reciprocal(recip,
                                         osb[half][64:65, base:base + 1024])
                    bsb = ptile.tile([64, 1024], F32, tag="bsb", bufs=4,
                                     name=f"bsb{h}{seg}_{rep}")
                    nc.gpsimd.partition_broadcast(bsb, recip)
                    o_re = osb[half][0:64, base:base + 1024].rearrange(
                        "p (r c) -> p c r", c=16)
                    b_re = bsb.rearrange("p (r c) -> p c r", c=16)
                    for t in range(8):
                        for h2 in range(2):
                            c = 2 * t + h2
                            nc.vector.tensor_tensor(
                                xT2[64 * h2:64 * h2 + 64, t,
                                    128 * h + 64 * seg:
                                    128 * h + 64 * seg + 64],
                                b_re[:, c, :], o_re[:, c, :],
                                op=mybir.AluOpType.mult)
                    if seg == 1:
                        for f in tail_fill[half]:
                            f()

        for f in fillers:
            f()

    def final_unit(st, ch):
            py = ps.tile([128, 512], F32, tag="P1", bufs=2,
                         name=f"py{st}{ch}_{rep}")
            # xT2-dependent matmul first so the psum slot isn't grabbed early
            for t in range(8):
                nc.tensor.matmul(py,
                                 xT2[:, t, bass.ts(st, 128)],
                                 w_sb["p"][:, t, bass.ts(ch, 512)],
                                 start=(t == 0), stop=False)
            nc.tensor.matmul(py, ones128[0:1, :],
                             bias_sb["p"][0:1, bass.ts(ch, 512)],
                             start=False, stop=True)
            ysb = ypool.tile([128, 512], F32, tag="y",
                             name=f"ysb{st}{ch}_{rep}")
            nc.scalar.copy(ysb, py)
            nc.sync.dma_start(out=y[bass.ts(st, 128), bass.ts(ch, 512)],
                                in_=ysb)

    def final(st):
        for ch in range(2):
            final_unit(st, ch)

    # pipeline: proj st0/st1 dense; pair-0 attention with proj st2/st3 as
    # PE fillers; pair-1 attention with final st0/st1 as fillers; tail.
    _xh_cache.clear()
    if phases < 2:
        if load_inputs is not None:
            load_inputs("v")
        for st in range(4):
            project(st)
        return
    # q/k projections of tiles 0/1 first so pair-0 transposes start early
    for st, nm in ((0, "q"), (1, "q"), (0, "k"), (1, "k")):
        for ch in range(2):
            project_unit(st, nm, ch)
    if load_inputs is not None:
        load_inputs("v")
    loaded0 = attend_load(0)
    for st in (0, 1):
        for ch in range(2):
            project_unit(st, "v", ch)
    loaded1_box = {}
    fill0 = [
        (lambda st=st, nm=nm, ch=ch: project_unit(st, nm, ch))
        for nm in ("q", "k") for st in (2, 3) for ch in range(2)
    ] + [
        lambda: loaded1_box.update(v=attend_load(1))
    ] + [
        (lambda st=st, ch=ch: project_unit(st, "v", ch))
        for st in (2, 3) for ch in range(2)
    ]
    attend(0, loaded0, fill0)
    if load_inputs is not None:
        load_inputs("p")
    loaded1 = loaded1_box["v"]
    if phases >= 3:
        fill1 = [
            (lambda st=st, ch=ch: final_unit(st, ch))
            for st in (0, 1) for ch in range(2)
        ]
        tails = ([(lambda ch=ch: final_unit(2, ch)) for ch in range(2)],
                 [(lambda ch=ch: final_unit(3, ch)) for ch in range(2)])
    else:
        fill1, tails = [], ((), ())
    attend(1, loaded1, fill1, tail_fill=tails)


# ---------------------------------------------------------------------------
# host side
# ---------------------------------------------------------------------------

_CACHE = {}


def _prep_inputs(q, k, v, Wq, bq, Wk, bk, Wv, bv, Wp, bp):
    scale = 1.0 / np.sqrt(64.0)
    wq_T = np.ascontiguousarray(np.asarray(Wq, np.float32).T).astype(np.float16)
    wk_T = np.ascontiguousarray(np.asarray(Wk, np.float32).T).astype(np.float16)
    wv_T = np.ascontiguousarray(np.asarray(Wv, np.float32).T).astype(np.float16)
    wp_T = np.ascontiguousarray(np.asarray(Wp, np.float32).T).astype(np.float16)
    shared = {
        "wqT": wq_T, "wkT": wk_T, "wvT": wv_T, "wpT": wp_T,
        "bq": (np.asarray(bq, np.float32) * scale).astype(np.float16)[None, :],
        "bk": np.asarray(bk, np.float16)[None, :],
        "bv": np.asarray(bv, np.float16)[None, :],
        "bp": np.asarray(bp, np.float16)[None, :],
    }
    in_maps = []
    for c in range(N_CORES):
        b, g = divmod(c, 4)
        rows = slice(SB * g, SB * (g + 1))
        m = dict(shared)
        m["qT"] = np.ascontiguousarray(
            np.asarray(q[b, rows], np.float32).T * scale).astype(np.float16)
        m["kT"] = np.ascontiguousarray(
            np.asarray(k[b, rows], np.float32).T).astype(np.float16)
        m["vT"] = np.ascontiguousarray(
            np.asarray(v[b, rows], np.float32).T).astype(np.float16)
        in_maps.append(m)
    return in_maps


def kernel(q, k, v, Wq, bq, Wk, bk, Wv, bv, Wp, bp):
    if "nc" not in _CACHE:
        _CACHE["nc"] = build()
    nc = _CACHE["nc"]
    in_maps = _prep_inputs(q, k, v, Wq, bq, Wk, bk, Wv, bv, Wp, bp)
    res = run_bass_kernel_spmd(nc, in_maps, core_ids=list(range(N_CORES)))
    out = np.empty((B, S, E), np.float32)
    for c in range(N_CORES):
        b, g = divmod(c, 4)
        out[b, SB * g:SB * (g + 1), :] = res.results[c]["y"]
    return out


if __name__ == "__main__":
    rng = np.random.default_rng(0)
    s = 1.0 / np.sqrt(E)
    ins = {
        "q": rng.standard_normal((B, S, E), dtype=np.float32),
        "k": rng.standard_normal((B, S, E), dtype=np.float32),
        "v": rng.standard_normal((B, S, E), dtype=np.float32),
        "Wq": rng.standard_normal((E, E), dtype=np.float32) * s,
        "bq": rng.standard_normal(E).astype(np.float32) * s,
        "Wk": rng.standard_normal((E, E), dtype=np.float32) * s,
        "bk": rng.standard_normal(E).astype(np.float32) * s,
        "Wv": rng.standard_normal((E, E), dtype=np.float32) * s,
        "bv": rng.standard_normal(E).astype(np.float32) * s,
        "Wp": rng.standard_normal((E, E), dtype=np.float32) * s,
        "bp": rng.standard_normal(E).astype(np.float32) * s,
    }
    out = kernel(**ins)
    print("kernel ran, out shape", out.shape, "mean", float(np.abs(out).mean()))


# revision 34
# speedup vs baseline: 1.0268x; 1.0268x over previous
"""Trainium2 Bass kernel for nn_MultiHeadAttention_66202625900642.

Reference semantics (B=2, S=2048, E=1024, H=16 heads, D=64):
    qh = q @ Wq.T + bq   (same k, v)
    head split is a PLAIN RESHAPE (B, S, E) -> (B, H, S, D):
      head h of batch b = rows [128h, 128h+128) of qh[b] reinterpreted
      row-major as a (2048, 64) matrix (scrambled seq index s' = 16r + c).
    causal softmax over s', out @ Wp.T + bp.

Because the head split partitions the *sequence* rows, sharding each batch
into 4 row-blocks of 512 (= 4 heads) is fully local: 8 cores = 2 batches x 4
quarters, zero collectives. Weights are replicated (fp16).

Per-core pipeline (all matmuls fp16, fp32 PSUM accumulation):
  1. projections -> qh/kh/vh fp16 (weights streamed per e-tile)
  2. DRAM round-trip: qh/kh into per-pair [2048, 128] files (2 heads wide),
     vh natural; DMA-transpose pair files back as [128, 2048] = two heads'
     Q_hT/K_hT stacked; vh re-read as [128, 65] V' tiles (ones column -> row
     sums ride along the P^T @ V' matmul).
  3. attention per head pair, both heads' S^T blocks issued to disjoint PE
     row groups (K=64 each -> concurrent on the 128x128 array): one exp per
     psum group on ACT, causal triangles via gpsimd affine_select,
     P^T @ V' accumulates out^T[d, s'] + rowsum into per-chunk PSUM,
     evacuated to SBUF.
  4. normalization: reciprocal of rowsum, PE-broadcast (f32r K=1 matmul),
     fused into the stride-16 rearrange to final-projection layout.
  5. final projection -> y fp32.
"""

import numpy as np

import concourse.bass as bass
import concourse.mybir as mybir
import concourse.tile as tile
from concourse import bacc
from concourse.bass_utils import run_bass_kernel_spmd

F16 = mybir.dt.float16
F32 = mybir.dt.float32
F32R = mybir.dt.float32r
EXP = mybir.ActivationFunctionType.Exp

B, S, E = 2, 2048, 1024
SB = 512                # seq rows per core (= 4 heads)
N_CORES = 8


def build(reps: int = 1, phases: int = 3):
    nc = bacc.Bacc(None, target_bir_lowering=False)

    qT = nc.dram_tensor("qT", [E, SB], F16, kind="ExternalInput")
    kT = nc.dram_tensor("kT", [E, SB], F16, kind="ExternalInput")
    vT = nc.dram_tensor("vT", [E, SB], F16, kind="ExternalInput")
    wqT = nc.dram_tensor("wqT", [E, E], F16, kind="ExternalInput")
    wkT = nc.dram_tensor("wkT", [E, E], F16, kind="ExternalInput")
    wvT = nc.dram_tensor("wvT", [E, E], F16, kind="ExternalInput")
    wpT = nc.dram_tensor("wpT", [E, E], F16, kind="ExternalInput")
    bq = nc.dram_tensor("bq", [1, E], F16, kind="ExternalInput")
    bk = nc.dram_tensor("bk", [1, E], F16, kind="ExternalInput")
    bv = nc.dram_tensor("bv", [1, E], F16, kind="ExternalInput")
    bp = nc.dram_tensor("bp", [1, E], F16, kind="ExternalInput")
    y = nc.dram_tensor("y", [SB, E], F32, kind="ExternalOutput")

    with tile.TileContext(nc) as tc:
        with (
            tc.tile_pool(name="consts", bufs=1) as consts,
            tc.tile_pool(name="wpool", bufs=1) as wpool,
            tc.tile_pool(name="proj", bufs=2) as proj,
            tc.tile_pool(name="attn", bufs=1) as attn,
            tc.tile_pool(name="ptile", bufs=3) as ptile,
            tc.tile_pool(name="ypool", bufs=2) as ypool,
            tc.tile_pool(name="ps", bufs=3, space="PSUM") as ps,
            tc.tile_pool(name="dram", bufs=1, space="DRAM") as dram,
        ):
            # ---- constants -------------------------------------------------
            ones128 = consts.tile([1, 128], F16)
            nc.vector.memset(ones128, 1.0)
            bias_sb = {}
            for nm, t in (("q", bq), ("k", bk), ("v", bv), ("p", bp)):
                b_t = consts.tile([1, E], F16, name=f"bias_{nm}")
                nc.sync.dma_start(out=b_t, in_=t[:, :])
                bias_sb[nm] = b_t

            # ---- weight/activation tiles; q/k loaded now, v/p deferred -----
            w_sb, x_sb, dram_in = {}, {}, {}
            for nm, wt, xt in (("q", wqT, qT), ("k", wkT, kT), ("v", wvT, vT)):
                w_t = wpool.tile([128, 8, E], F16, name=f"w_{nm}")
                x_t = wpool.tile([128, 8, SB], F16, name=f"x_{nm}")
                dram_in[nm] = (wt, xt)
                w_sb[nm], x_sb[nm] = w_t, x_t
            w_p = wpool.tile([128, 8, E], F16, name="w_p")
            w_sb["p"] = w_p
            dram_in["p"] = (wpT, None)

            def load_inputs(nm, eng=None):
                eng = eng or nc.sync
                wt, xt = dram_in[nm]
                wre = wt.ap().rearrange("(t p) f -> p t f", p=128)
                if xt is not None:
                    xre = xt.ap().rearrange("(t p) s -> p t s", p=128)
                    eng.dma_start(out=x_sb[nm], in_=xre)
                for t2 in range(2):
                    eng.dma_start(out=w_sb[nm][:, 4 * t2:4 * t2 + 4],
                                  in_=wre[:, 4 * t2:4 * t2 + 4])

            load_inputs("q")
            load_inputs("k")

            # ---- DRAM scratch ---------------------------------------------
            qkp = [dram.tile([2 * S, 128], F16, name=f"qkp{i}")
                   for i in range(2)]
            vh_d = dram.tile([SB, E], F16)

            for rep in range(reps):
                _body(nc, tc, ps, proj, attn, ptile, ypool,
                      ones128, bias_sb, w_sb, x_sb, qkp, vh_d, y,
                      rep, phases, load_inputs if rep == 0 else None)
    nc.finalize()
    return nc


def _body(nc, tc, ps, proj, attn, ptile, ypool, ones128,
          bias_sb, w_sb, x_sb, qkp, vh_d, y, rep, phases=3,
          load_inputs=None):
    xT2 = attn.tile([128, 8, SB], F16, tag="xT2", name=f"xT2_{rep}")
    if phases < 2:
        nc.vector.memset(xT2[:, 0, 0:1], 0.0)
    _xh_cache = {}

    def project_unit(st, nm, ch):
        # one psum-group of the projection for (seq-tile st, proj nm, chunk ch)
        xh = _xh_cache.get((st, nm))
        if xh is None:
            xh = proj.tile([128, E], F16, tag="xh", name=f"xh_{nm}{st}_{rep}")
            _xh_cache[(st, nm)] = xh
        pp = ps.tile([128, 512], F32, tag="P1", bufs=2, name=f"pp{rep}")
        nc.tensor.matmul(pp, ones128[0:1, :],
                         bias_sb[nm][0:1, bass.ts(ch, 512)],
                         start=True, stop=False)
        for t in range(8):
            nc.tensor.matmul(
                pp,
                x_sb[nm][:, t, bass.ts(st, 128)],
                w_sb[nm][:, t, bass.ts(ch, 512)],
                start=False, stop=(t == 7))
        nc.vector.tensor_copy(xh[:, bass.ts(ch, 512)], pp)
        if ch == 1:
            if nm == "v":
                nc.sync.dma_start(out=vh_d[bass.ts(st, 128), :], in_=xh)
            else:
                tgt = qkp[st // 2]
                base = (0 if nm == "q" else S * 128) + 64 * (st % 2)
                out_ap = bass.AP(
                    tgt.tensor, tgt.offset + base,
                    [[2048, 128], [128, 16], [1, 64]])
                nc.sync.dma_start(
                    out=out_ap, in_=xh.rearrange("r (c d) -> r c d", d=64))

    def project(st):
        for nm in ("q", "k", "v"):
            for ch in range(2):
                project_unit(st, nm, ch)

    def attend_load(pair):
        QKT = ptile.tile([128, 2 * S], F16, tag="QKT", bufs=2,
                         name=f"QKT{pair}_{rep}")
        nc.scalar.dma_start(out=QKT, in_=qkp[pair][:, :], transpose=True)
        return QKT[:, 0:S], QKT[:, S:2 * S]

    def attend(pair, loaded, fillers=(), tail_fill=((), ())):
        QT, KT = loaded
        fillers = list(fillers)
        vps = []
        for half in range(2):
            h = 2 * pair + half
            vp = ptile.tile([128, 16, 65], F16, tag="vp", bufs=4,
                            name=f"vp{h}_{rep}")
            v_src = bass.AP(vh_d.tensor, vh_d.offset + 128 * h * E,
                            [[64, 128], [8192, 16], [1, 64]])
            nc.sync.dma_start(out=vp[:, :, 0:64], in_=v_src)
            nc.vector.memset(vp[:, :, 64:65], 1.0)
            vps.append(vp)

        # per-head SBUF accumulators for out^T (+rowsum row 64)
        osb = [ptile.tile([65, 2048], F32, tag="osb", bufs=3,
                          name=f"osb{2 * pair + half}_{rep}")
               for half in range(2)]

        LAG = 2   # defer V-matmuls 2 groups behind S^T/exp (pt bufs cover it)
        pending = []

        def emit_vmms(ent):
            qc_, js_, pts_, psO_ = ent
            jmax_ = 4 * qc_ + 3
            for half in range(2):
                pt = pts_[half]
                for jj, j in enumerate(js_):
                    o = j - 4 * qc_
                    lo = 0 if o < 0 else 128 * o
                    nc.tensor.matmul(
                        psO_[half][:, lo:],
                        vps[half][:, j, :],
                        pt[:, 512 * jj + lo:512 * jj + 512],
                        start=(j == 0), stop=(j == jmax_))

        for qc in (2, 3, 0, 1):
            jmax = 4 * qc + 3
            psO = [ps.tile([65, 512], F32, tag="O", bufs=2,
                           name=f"psO{2 * pair + half}_{qc}_{rep}")
                   for half in range(2)]
            for j0 in range(0, jmax + 1, 2):
                js = [j for j in (j0, j0 + 1) if j <= jmax]
                lo0 = max(0, 128 * (js[0] - 4 * qc))
                pts = []
                for half in range(2):
                    psS = ps.tile([128, 1024], F32, tag="S", bufs=2,
                                  name=f"psS{half}_{qc}_{j0}_{rep}")
                    pt = ptile.tile([128, 1024], F16, tag="P", bufs=4,
                                    name=f"pt{half}_{qc}_{j0}_{rep}")
                    r0, r1 = 64 * half, 64 * half + 64
                    for jj, j in enumerate(js):
                        o = j - 4 * qc
                        lo = 0 if o < 0 else 128 * o
                        nc.tensor.matmul(
                            psS[:, 512 * jj + lo:512 * jj + 512],
                            KT[r0:r1, bass.ts(j, 128)],
                            QT[r0:r1, 512 * qc + lo:512 * qc + 512],
                            start=True, stop=True)
                    # one exp per group; stale lead-in cols are never read
                    nc.scalar.activation(pt[:, lo0:], psS[:, lo0:], EXP)
                    pts.append(pt)
                    for jj, j in enumerate(js):
                        o = j - 4 * qc
                        if o >= 0:
                            sl = pts[half][:, 512 * jj + 128 * o:
                                           512 * jj + 128 * o + 128]
                            nc.gpsimd.affine_select(
                                out=sl, in_=sl,
                                pattern=[[1, 128]],
                                compare_op=mybir.AluOpType.is_ge,
                                fill=0.0, base=0, channel_multiplier=-1)
                if fillers:
                    fillers.pop(0)()   # independent PE work while exp runs
                pending.append((qc, js, pts, psO))
                if len(pending) > LAG:
                    emit_vmms(pending.pop(0))
                if fillers:
                    fillers.pop(0)()
            # drain this qc's V-matmuls before evacuating its psO
            while pending:
                emit_vmms(pending.pop(0))
            for half in range(2):
                nc.vector.tensor_copy(osb[half][:, bass.ts(qc, 512)],
                                      psO[half])

            if qc in (3, 1):
                # normalize the finished s' segment [1024*seg, 1024*(seg+1))
                seg = qc // 2
                base = 1024 * seg
                for half in range(2):
                    h = 2 * pair + half
                    recip = ptile.tile([1, 1024], F32, tag="recip", bufs=4,
                                       name=f"recip{h}{seg}_{rep}")
                    nc.vector.reciprocal(recip,
                                         osb[half][64:65, base:base + 1024])
                    bsb = ptile.tile([64, 1024], F32, tag="bsb", bufs=4,
                                     name=f"bsb{h}{seg}_{rep}")
                    nc.gpsimd.partition_broadcast(bsb, recip)
                    o_re = osb[half][0:64, base:base + 1024].rearrange(
                        "p (r c) -> p c r", c=16)
                    b_re = bsb.rearrange("p (r c) -> p c r", c=16)
                    for t in range(8):
                        for h2 in range(2):
                            c = 2 * t + h2
                            nc.vector.tensor_tensor(
                                xT2[64 * h2:64 * h2 + 64, t,
                                    128 * h + 64 * seg:
                                    128 * h + 64 * seg + 64],
                                b_re[:, c, :], o_re[:, c, :],
                                op=mybir.AluOpType.mult)
                    if seg == 0:   # last norm segment in (2,3,0,1) order
                        for f in tail_fill[half]:
                            f()

        for f in fillers:
            f()

    def final_unit(st, ch):
            py = ps.tile([128, 512], F32, tag="P1", bufs=2,
                         name=f"py{st}{ch}_{rep}")
            # xT2-dependent matmul first so the psum slot isn't grabbed early
            for t in range(8):
                nc.tensor.matmul(py,
                                 xT2[:, t, bass.ts(st, 128)],
                                 w_sb["p"][:, t, bass.ts(ch, 512)],
                                 start=(t == 0), stop=False)
            nc.tensor.matmul(py, ones128[0:1, :],
                             bias_sb["p"][0:1, bass.ts(ch, 512)],
                             start=False, stop=True)
            ysb = ypool.tile([128, 512], F32, tag="y",
                             name=f"ysb{st}{ch}_{rep}")
            nc.scalar.copy(ysb, py)
            nc.sync.dma_start(out=y[bass.ts(st, 128), bass.ts(ch, 512)],
                                in_=ysb)

    def final(st):
        for ch in range(2):
            final_unit(st, ch)

    # pipeline: proj st0/st1 dense; pair-0 attention with proj st2/st3 as
    # PE fillers; pair-1 attention with final st0/st1 as fillers; tail.
    _xh_cache.clear()
    if phases < 2:
        if load_inputs is not None:
            load_inputs("v")
        for st in range(4):
            project(st)
        return
    # q/k projections of tiles 0/1 first so pair-0 transposes start early
    for st, nm in ((0, "q"), (1, "q"), (0, "k"), (1, "k")):
        for ch in range(2):
            project_unit(st, nm, ch)
    loaded0 = attend_load(0)
    if load_inputs is not None:
        load_inputs("v", nc.scalar)
    for st in (0, 1):
        for ch in range(2):
            project_unit(st, "v", ch)
    loaded1_box = {}
    fill0 = [
        (lambda st=st, nm=nm, ch=ch: project_unit(st, nm, ch))
        for nm in ("q", "k") for st in (2, 3) for ch in range(2)
    ] + [
        lambda: loaded1_box.update(v=attend_load(1))
    ] + [
        (lambda st=st, ch=ch: project_unit(st, "v", ch))
        for st in (2, 3) for ch in range(2)
    ]
    attend(0, loaded0, fill0)
    if load_inputs is not None:
        load_inputs("p")
    loaded1 = loaded1_box["v"]
    if phases >= 3:
        fill1 = [
            (lambda st=st, ch=ch: final_unit(st, ch))
            for st in (0, 1) for ch in range(2)
        ]
        tails = ([(lambda ch=ch: final_unit(2, ch)) for ch in range(2)],
                 [(lambda ch=ch: final_unit(3, ch)) for ch in range(2)])
    else:
        fill1, tails = [], ((), ())
    attend(1, loaded1, fill1, tail_fill=tails)


# ---------------------------------------------------------------------------
# host side
# ---------------------------------------------------------------------------

_CACHE = {}


def _prep_inputs(q, k, v, Wq, bq, Wk, bk, Wv, bv, Wp, bp):
    scale = 1.0 / np.sqrt(64.0)
    wq_T = np.ascontiguousarray(np.asarray(Wq, np.float32).T).astype(np.float16)
    wk_T = np.ascontiguousarray(np.asarray(Wk, np.float32).T).astype(np.float16)
    wv_T = np.ascontiguousarray(np.asarray(Wv, np.float32).T).astype(np.float16)
    wp_T = np.ascontiguousarray(np.asarray(Wp, np.float32).T).astype(np.float16)
    shared = {
        "wqT": wq_T, "wkT": wk_T, "wvT": wv_T, "wpT": wp_T,
        "bq": (np.asarray(bq, np.float32) * scale).astype(np.float16)[None, :],
        "bk": np.asarray(bk, np.float16)[None, :],
        "bv": np.asarray(bv, np.float16)[None, :],
        "bp": np.asarray(bp, np.float16)[None, :],
    }
    in_maps = []
    for c in range(N_CORES):
        b, g = divmod(c, 4)
        rows = slice(SB * g, SB * (g + 1))
        m = dict(shared)
        m["qT"] = np.ascontiguousarray(
            np.asarray(q[b, rows], np.float32).T * scale).astype(np.float16)
        m["kT"] = np.ascontiguousarray(
            np.asarray(k[b, rows], np.float32).T).astype(np.float16)
        m["vT"] = np.ascontiguousarray(
            np.asarray(v[b, rows], np.float32).T).astype(np.float16)
        in_maps.append(m)
    return in_maps


def kernel(q, k, v, Wq, bq, Wk, bk, Wv, bv, Wp, bp):
    if "nc" not in _CACHE:
        _CACHE["nc"] = build()
    nc = _CACHE["nc"]
    in_maps = _prep_inputs(q, k, v, Wq, bq, Wk, bk, Wv, bv, Wp, bp)
    res = run_bass_kernel_spmd(nc, in_maps, core_ids=list(range(N_CORES)))
    out = np.empty((B, S, E), np.float32)
    for c in range(N_CORES):
        b, g = divmod(c, 4)
        out[b, SB * g:SB * (g + 1), :] = res.results[c]["y"]
    return out


if __name__ == "__main__":
    rng = np.random.default_rng(0)
    s = 1.0 / np.sqrt(E)
    ins = {
        "q": rng.standard_normal((B, S, E), dtype=np.float32),
        "k": rng.standard_normal((B, S, E), dtype=np.float32),
        "v": rng.standard_normal((B, S, E), dtype=np.float32),
        "Wq": rng.standard_normal((E, E), dtype=np.float32) * s,
        "bq": rng.standard_normal(E).astype(np.float32) * s,
        "Wk": rng.standard_normal((E, E), dtype=np.float32) * s,
        "bk": rng.standard_normal(E).astype(np.float32) * s,
        "Wv": rng.standard_normal((E, E), dtype=np.float32) * s,
        "bv": rng.standard_normal(E).astype(np.float32) * s,
        "Wp": rng.standard_normal((E, E), dtype=np.float32) * s,
        "bp": rng.standard_normal(E).astype(np.float32) * s,
    }
    out = kernel(**ins)
    print("kernel ran, out shape", out.shape, "mean", float(np.abs(out).mean()))


# revision 37
# speedup vs baseline: 1.0601x; 1.0323x over previous
"""Trainium2 Bass kernel for nn_MultiHeadAttention_66202625900642.

Reference semantics (B=2, S=2048, E=1024, H=16 heads, D=64):
    qh = q @ Wq.T + bq   (same k, v)
    head split is a PLAIN RESHAPE (B, S, E) -> (B, H, S, D):
      head h of batch b = rows [128h, 128h+128) of qh[b] reinterpreted
      row-major as a (2048, 64) matrix (scrambled seq index s' = 16r + c).
    causal softmax over s', out @ Wp.T + bp.

Because the head split partitions the *sequence* rows, sharding each batch
into 4 row-blocks of 512 (= 4 heads) is fully local: 8 cores = 2 batches x 4
quarters, zero collectives. Weights are replicated (fp16).

Per-core pipeline (all matmuls fp16, fp32 PSUM accumulation):
  1. projections -> qh/kh/vh fp16 (weights streamed per e-tile)
  2. DRAM round-trip: qh/kh into per-pair [2048, 128] files (2 heads wide),
     vh natural; DMA-transpose pair files back as [128, 2048] = two heads'
     Q_hT/K_hT stacked; vh re-read as [128, 65] V' tiles (ones column -> row
     sums ride along the P^T @ V' matmul).
  3. attention per head pair, both heads' S^T blocks issued to disjoint PE
     row groups (K=64 each -> concurrent on the 128x128 array): one exp per
     psum group on ACT, causal triangles via gpsimd affine_select,
     P^T @ V' accumulates out^T[d, s'] + rowsum into per-chunk PSUM,
     evacuated to SBUF.
  4. normalization: reciprocal of rowsum, PE-broadcast (f32r K=1 matmul),
     fused into the stride-16 rearrange to final-projection layout.
  5. final projection -> y fp32.
"""

import numpy as np

import concourse.bass as bass
import concourse.mybir as mybir
import concourse.tile as tile
from concourse import bacc
from concourse.bass_utils import run_bass_kernel_spmd

F16 = mybir.dt.float16
F32 = mybir.dt.float32
F32R = mybir.dt.float32r
EXP = mybir.ActivationFunctionType.Exp

B, S, E = 2, 2048, 1024
SB = 512                # seq rows per core (= 4 heads)
N_CORES = 8


def build(reps: int = 1, phases: int = 3):
    nc = bacc.Bacc(None, target_bir_lowering=False)

    qT = nc.dram_tensor("qT", [E, SB], F16, kind="ExternalInput")
    kT = nc.dram_tensor("kT", [E, SB], F16, kind="ExternalInput")
    vT = nc.dram_tensor("vT", [E, SB], F16, kind="ExternalInput")
    wqT = nc.dram_tensor("wqT", [E, E], F16, kind="ExternalInput")
    wkT = nc.dram_tensor("wkT", [E, E], F16, kind="ExternalInput")
    wvT = nc.dram_tensor("wvT", [E, E], F16, kind="ExternalInput")
    wpT = nc.dram_tensor("wpT", [E, E], F16, kind="ExternalInput")
    bq = nc.dram_tensor("bq", [1, E], F16, kind="ExternalInput")
    bk = nc.dram_tensor("bk", [1, E], F16, kind="ExternalInput")
    bv = nc.dram_tensor("bv", [1, E], F16, kind="ExternalInput")
    bp = nc.dram_tensor("bp", [1, E], F16, kind="ExternalInput")
    y = nc.dram_tensor("y", [SB, E], F32, kind="ExternalOutput")

    with tile.TileContext(nc) as tc:
        with (
            tc.tile_pool(name="consts", bufs=1) as consts,
            tc.tile_pool(name="wpool", bufs=1) as wpool,
            tc.tile_pool(name="proj", bufs=2) as proj,
            tc.tile_pool(name="attn", bufs=1) as attn,
            tc.tile_pool(name="ptile", bufs=3) as ptile,
            tc.tile_pool(name="ypool", bufs=2) as ypool,
            tc.tile_pool(name="ps", bufs=3, space="PSUM") as ps,
            tc.tile_pool(name="dram", bufs=1, space="DRAM") as dram,
        ):
            # ---- constants -------------------------------------------------
            ones128 = consts.tile([1, 128], F16)
            nc.vector.memset(ones128, 1.0)
            bias_sb = {}
            for nm, t in (("q", bq), ("k", bk), ("v", bv), ("p", bp)):
                b_t = consts.tile([1, E], F16, name=f"bias_{nm}")
                nc.sync.dma_start(out=b_t, in_=t[:, :])
                bias_sb[nm] = b_t

            # ---- weight/activation tiles; q/k loaded now, v/p deferred -----
            w_sb, x_sb, dram_in = {}, {}, {}
            for nm, wt, xt in (("q", wqT, qT), ("k", wkT, kT), ("v", wvT, vT)):
                w_t = wpool.tile([128, 8, E], F16, name=f"w_{nm}")
                x_t = wpool.tile([128, 8, SB], F16, name=f"x_{nm}")
                dram_in[nm] = (wt, xt)
                w_sb[nm], x_sb[nm] = w_t, x_t
            w_p = wpool.tile([128, 8, E], F16, name="w_p")
            w_sb["p"] = w_p
            dram_in["p"] = (wpT, None)

            def load_inputs(nm, eng=None):
                eng = eng or nc.sync
                wt, xt = dram_in[nm]
                wre = wt.ap().rearrange("(t p) f -> p t f", p=128)
                if xt is not None:
                    xre = xt.ap().rearrange("(t p) s -> p t s", p=128)
                    eng.dma_start(out=x_sb[nm], in_=xre)
                for t2 in range(2):
                    eng.dma_start(out=w_sb[nm][:, 4 * t2:4 * t2 + 4],
                                  in_=wre[:, 4 * t2:4 * t2 + 4])

            load_inputs("q")
            load_inputs("k")

            # ---- DRAM scratch ---------------------------------------------
            qkp = [dram.tile([2 * S, 128], F16, name=f"qkp{i}")
                   for i in range(2)]
            vh_d = dram.tile([SB, E], F16)

            for rep in range(reps):
                _body(nc, tc, ps, proj, attn, ptile, ypool,
                      ones128, bias_sb, w_sb, x_sb, qkp, vh_d, y,
                      rep, phases, load_inputs if rep == 0 else None)
    nc.finalize()
    return nc


def _body(nc, tc, ps, proj, attn, ptile, ypool, ones128,
          bias_sb, w_sb, x_sb, qkp, vh_d, y, rep, phases=3,
          load_inputs=None):
    xT2 = attn.tile([128, 8, SB], F16, tag="xT2", name=f"xT2_{rep}")
    if phases < 2:
        nc.vector.memset(xT2[:, 0, 0:1], 0.0)
    _xh_cache = {}

    def project_unit(st, nm, ch):
        # one psum-group of the projection for (seq-tile st, proj nm, chunk ch)
        xh = _xh_cache.get((st, nm))
        if xh is None:
            xh = proj.tile([128, E], F16, tag="xh", name=f"xh_{nm}{st}_{rep}")
            _xh_cache[(st, nm)] = xh
        pp = ps.tile([128, 512], F32, tag="P1", bufs=2, name=f"pp{rep}")
        nc.tensor.matmul(pp, ones128[0:1, :],
                         bias_sb[nm][0:1, bass.ts(ch, 512)],
                         start=True, stop=False)
        for t in range(8):
            nc.tensor.matmul(
                pp,
                x_sb[nm][:, t, bass.ts(st, 128)],
                w_sb[nm][:, t, bass.ts(ch, 512)],
                start=False, stop=(t == 7))
        nc.vector.tensor_copy(xh[:, bass.ts(ch, 512)], pp)
        if ch == 1:
            if nm == "v":
                nc.sync.dma_start(out=vh_d[bass.ts(st, 128), :], in_=xh)
            else:
                tgt = qkp[st // 2]
                base = (0 if nm == "q" else S * 128) + 64 * (st % 2)
                out_ap = bass.AP(
                    tgt.tensor, tgt.offset + base,
                    [[2048, 128], [128, 16], [1, 64]])
                nc.sync.dma_start(
                    out=out_ap, in_=xh.rearrange("r (c d) -> r c d", d=64))

    def project(st):
        for nm in ("q", "k", "v"):
            for ch in range(2):
                project_unit(st, nm, ch)

    def attend_load(pair):
        QKT = ptile.tile([128, 2 * S], F16, tag="QKT", bufs=2,
                         name=f"QKT{pair}_{rep}")
        nc.scalar.dma_start(out=QKT, in_=qkp[pair][:, :], transpose=True)
        return QKT[:, 0:S], QKT[:, S:2 * S]

    def attend(pair, loaded, fillers=(), tail_fill=((), ())):
        QT, KT = loaded
        fillers = list(fillers)
        vps = []
        for half in range(2):
            h = 2 * pair + half
            vp = ptile.tile([128, 16, 65], F16, tag="vp", bufs=4,
                            name=f"vp{h}_{rep}")
            v_src = bass.AP(vh_d.tensor, vh_d.offset + 128 * h * E,
                            [[64, 128], [8192, 16], [1, 64]])
            nc.sync.dma_start(out=vp[:, :, 0:64], in_=v_src)
            nc.vector.memset(vp[:, :, 64:65], 1.0)
            vps.append(vp)

        # per-head SBUF accumulators for out^T (+rowsum row 64)
        osb = [ptile.tile([65, 2048], F32, tag="osb", bufs=3,
                          name=f"osb{2 * pair + half}_{rep}")
               for half in range(2)]

        LAG = 2   # defer V-matmuls 2 groups behind S^T/exp (pt bufs cover it)
        pending = []

        def emit_vmms(ent):
            qc_, js_, pts_, psO_ = ent
            jmax_ = 4 * qc_ + 3
            for half in range(2):
                pt = pts_[half]
                for jj, j in enumerate(js_):
                    o = j - 4 * qc_
                    lo = 0 if o < 0 else 128 * o
                    nc.tensor.matmul(
                        psO_[half][:, lo:],
                        vps[half][:, j, :],
                        pt[:, 512 * jj + lo:512 * jj + 512],
                        start=(j == 0), stop=(j == jmax_))

        for qc in (1, 0, 2, 3):
            jmax = 4 * qc + 3
            psO = [ps.tile([65, 512], F32, tag="O", bufs=2,
                           name=f"psO{2 * pair + half}_{qc}_{rep}")
                   for half in range(2)]
            for j0 in range(0, jmax + 1, 2):
                js = [j for j in (j0, j0 + 1) if j <= jmax]
                lo0 = max(0, 128 * (js[0] - 4 * qc))
                pts = []
                for half in range(2):
                    psS = ps.tile([128, 1024], F32, tag="S", bufs=2,
                                  name=f"psS{half}_{qc}_{j0}_{rep}")
                    pt = ptile.tile([128, 1024], F16, tag="P", bufs=4,
                                    name=f"pt{half}_{qc}_{j0}_{rep}")
                    r0, r1 = 64 * half, 64 * half + 64
                    for jj, j in enumerate(js):
                        o = j - 4 * qc
                        lo = 0 if o < 0 else 128 * o
                        nc.tensor.matmul(
                            psS[:, 512 * jj + lo:512 * jj + 512],
                            KT[r0:r1, bass.ts(j, 128)],
                            QT[r0:r1, 512 * qc + lo:512 * qc + 512],
                            start=True, stop=True)
                    # one exp per group; stale lead-in cols are never read
                    nc.scalar.activation(pt[:, lo0:], psS[:, lo0:], EXP)
                    pts.append(pt)
                    for jj, j in enumerate(js):
                        o = j - 4 * qc
                        if o >= 0:
                            sl = pts[half][:, 512 * jj + 128 * o:
                                           512 * jj + 128 * o + 128]
                            nc.gpsimd.affine_select(
                                out=sl, in_=sl,
                                pattern=[[1, 128]],
                                compare_op=mybir.AluOpType.is_ge,
                                fill=0.0, base=0, channel_multiplier=-1)
                if fillers:
                    fillers.pop(0)()   # independent PE work while exp runs
                pending.append((qc, js, pts, psO))
                if len(pending) > LAG:
                    emit_vmms(pending.pop(0))
                if fillers:
                    fillers.pop(0)()
            # drain this qc's V-matmuls before evacuating its psO
            while pending:
                emit_vmms(pending.pop(0))
            for half in range(2):
                nc.vector.tensor_copy(osb[half][:, bass.ts(qc, 512)],
                                      psO[half])

            if qc in (0, 3):
                # normalize the finished s' segment [1024*seg, 1024*(seg+1))
                seg = 0 if qc == 0 else 1
                base = 1024 * seg
                for half in range(2):
                    h = 2 * pair + half
                    recip = ptile.tile([1, 1024], F32, tag="recip", bufs=4,
                                       name=f"recip{h}{seg}_{rep}")
                    nc.vector.reciprocal(recip,
                                         osb[half][64:65, base:base + 1024])
                    bsb = ptile.tile([64, 1024], F32, tag="bsb", bufs=4,
                                     name=f"bsb{h}{seg}_{rep}")
                    nc.gpsimd.partition_broadcast(bsb, recip)
                    o_re = osb[half][0:64, base:base + 1024].rearrange(
                        "p (r c) -> p c r", c=16)
                    b_re = bsb.rearrange("p (r c) -> p c r", c=16)
                    for t in range(8):
                        for h2 in range(2):
                            c = 2 * t + h2
                            nc.vector.tensor_tensor(
                                xT2[64 * h2:64 * h2 + 64, t,
                                    128 * h + 64 * seg:
                                    128 * h + 64 * seg + 64],
                                b_re[:, c, :], o_re[:, c, :],
                                op=mybir.AluOpType.mult)
                    if seg == 1:
                        for f in tail_fill[half]:
                            f()

        for f in fillers:
            f()

    def final_unit(st, ch):
            py = ps.tile([128, 512], F32, tag="P1", bufs=2,
                         name=f"py{st}{ch}_{rep}")
            # xT2-dependent matmul first so the psum slot isn't grabbed early
            for t in range(8):
                nc.tensor.matmul(py,
                                 xT2[:, t, bass.ts(st, 128)],
                                 w_sb["p"][:, t, bass.ts(ch, 512)],
                                 start=(t == 0), stop=False)
            nc.tensor.matmul(py, ones128[0:1, :],
                             bias_sb["p"][0:1, bass.ts(ch, 512)],
                             start=False, stop=True)
            ysb = ypool.tile([128, 512], F32, tag="y",
                             name=f"ysb{st}{ch}_{rep}")
            nc.scalar.copy(ysb, py)
            nc.sync.dma_start(out=y[bass.ts(st, 128), bass.ts(ch, 512)],
                                in_=ysb)

    def final(st):
        for ch in range(2):
            final_unit(st, ch)

    # pipeline: proj st0/st1 dense; pair-0 attention with proj st2/st3 as
    # PE fillers; pair-1 attention with final st0/st1 as fillers; tail.
    _xh_cache.clear()
    if phases < 2:
        if load_inputs is not None:
            load_inputs("v")
        for st in range(4):
            project(st)
        return
    # q/k projections of tiles 0/1 first so pair-0 transposes start early
    for st, nm in ((0, "q"), (1, "q"), (0, "k"), (1, "k")):
        for ch in range(2):
            project_unit(st, nm, ch)
    loaded0 = attend_load(0)
    if load_inputs is not None:
        load_inputs("v", nc.scalar)
    for st in (0, 1):
        for ch in range(2):
            project_unit(st, "v", ch)
    loaded1_box = {}
    fill0 = [
        (lambda st=st, nm=nm, ch=ch: project_unit(st, nm, ch))
        for nm in ("q", "k") for st in (2, 3) for ch in range(2)
    ] + [
        lambda: loaded1_box.update(v=attend_load(1))
    ] + [
        (lambda st=st, ch=ch: project_unit(st, "v", ch))
        for st in (2, 3) for ch in range(2)
    ]
    attend(0, loaded0, fill0)
    if load_inputs is not None:
        load_inputs("p")
    loaded1 = loaded1_box["v"]
    if phases >= 3:
        fill1 = [
            (lambda st=st, ch=ch: final_unit(st, ch))
            for st in (0, 1) for ch in range(2)
        ]
        tails = ([(lambda ch=ch: final_unit(2, ch)) for ch in range(2)],
                 [(lambda ch=ch: final_unit(3, ch)) for ch in range(2)])
    else:
        fill1, tails = [], ((), ())
    attend(1, loaded1, fill1, tail_fill=tails)


# ---------------------------------------------------------------------------
# host side
# ---------------------------------------------------------------------------

_CACHE = {}


def _prep_inputs(q, k, v, Wq, bq, Wk, bk, Wv, bv, Wp, bp):
    scale = 1.0 / np.sqrt(64.0)
    wq_T = np.ascontiguousarray(np.asarray(Wq, np.float32).T).astype(np.float16)
    wk_T = np.ascontiguousarray(np.asarray(Wk, np.float32).T).astype(np.float16)
    wv_T = np.ascontiguousarray(np.asarray(Wv, np.float32).T).astype(np.float16)
    wp_T = np.ascontiguousarray(np.asarray(Wp, np.float32).T).astype(np.float16)
    shared = {
        "wqT": wq_T, "wkT": wk_T, "wvT": wv_T, "wpT": wp_T,
        "bq": (np.asarray(bq, np.float32) * scale).astype(np.float16)[None, :],
        "bk": np.asarray(bk, np.float16)[None, :],
        "bv": np.asarray(bv, np.float16)[None, :],
        "bp": np.asarray(bp, np.float16)[None, :],
    }
    in_maps = []
    for c in range(N_CORES):
        b, g = divmod(c, 4)
        rows = slice(SB * g, SB * (g + 1))
        m = dict(shared)
        m["qT"] = np.ascontiguousarray(
            np.asarray(q[b, rows], np.float32).T * scale).astype(np.float16)
        m["kT"] = np.ascontiguousarray(
            np.asarray(k[b, rows], np.float32).T).astype(np.float16)
        m["vT"] = np.ascontiguousarray(
            np.asarray(v[b, rows], np.float32).T).astype(np.float16)
        in_maps.append(m)
    return in_maps


def kernel(q, k, v, Wq, bq, Wk, bk, Wv, bv, Wp, bp):
    if "nc" not in _CACHE:
        _CACHE["nc"] = build()
    nc = _CACHE["nc"]
    in_maps = _prep_inputs(q, k, v, Wq, bq, Wk, bk, Wv, bv, Wp, bp)
    res = run_bass_kernel_spmd(nc, in_maps, core_ids=list(range(N_CORES)))
    out = np.empty((B, S, E), np.float32)
    for c in range(N_CORES):
        b, g = divmod(c, 4)
        out[b, SB * g:SB * (g + 1), :] = res.results[c]["y"]
    return out


if __name__ == "__main__":
    rng = np.random.default_rng(0)
    s = 1.0 / np.sqrt(E)
    ins = {
        "q": rng.standard_normal((B, S, E), dtype=np.float32),
        "k": rng.standard_normal((B, S, E), dtype=np.float32),
        "v": rng.standard_normal((B, S, E), dtype=np.float32),
        "Wq": rng.standard_normal((E, E), dtype=np.float32) * s,
        "bq": rng.standard_normal(E).astype(np.float32) * s,
        "Wk": rng.standard_normal((E, E), dtype=np.float32) * s,
        "bk": rng.standard_normal(E).astype(np.float32) * s,
        "Wv": rng.standard_normal((E, E), dtype=np.float32) * s,
        "bv": rng.standard_normal(E).astype(np.float32) * s,
        "Wp": rng.standard_normal((E, E), dtype=np.float32) * s,
        "bp": rng.standard_normal(E).astype(np.float32) * s,
    }
    out = kernel(**ins)
    print("kernel ran, out shape", out.shape, "mean", float(np.abs(out).mean()))
